# revision 14
# baseline (speedup 1.0000x reference)
"""Trainium2 Bass kernel for nn_DGN (graph attention network + GRU).

Data-parallel over batch: 32 batches -> 8 cores x 4 batches.
Layout strategy: all per-node activations kept transposed [H=128, N] so that
H (=128) sits on partitions. Scores are computed transposed ([keys, queries])
so the attention-weight matrix is directly usable as the moving operand of the
att@v matmul (contraction over keys on partitions) -- no per-layer transposes.
Softmax has no row-max subtraction (scores empirically in [0, ~5]); the row
sums come from a ones-weights matmul that also broadcasts them to all
partitions for free.  The mask is transposed per batch via 3D XBAR DMA in
bf16 (out[p, e, f] = in[f, e*128+p]).  GRU matmuls run in float32r.
"""

import os
import sys
import functools
import numpy as np
import ml_dtypes

sys.path.insert(0, "/opt/trn_rl_repo")

import concourse.bass as bass
import concourse.bacc as bacc
import concourse.tile as tile
import concourse.mybir as mybir
from concourse import bass_utils
from concourse.masks import make_identity

B, N, DIN, H, A = 32, 1000, 64, 128, 10
NCORES = 8
BPC = B // NCORES  # batches per core
NP = 1024          # padded node count
NT = 8             # 128-row tiles per batch
F = 512            # psum half (one bank of fp32)
H3 = 3 * H

f32 = mybir.dt.float32
f32r = mybir.dt.float32r
bf16 = mybir.dt.bfloat16
AF = mybir.ActivationFunctionType
OP = mybir.AluOpType

def _ts(it):
    """rows of tile it that hold real data (last tile ragged: 1000 = 7*128+104)"""
    return N - it * 128 if it == NT - 1 else 128


def build_program():
    nc = bacc.Bacc("TRN2", debug=False, num_devices=NCORES)

    # ---- DRAM I/O ----
    x_d = nc.dram_tensor("x", [BPC, N, DIN], f32, kind="ExternalInput").ap()
    mask_d = nc.dram_tensor("mask", [BPC, N, N], f32, kind="ExternalInput").ap()
    hid_d = nc.dram_tensor("hid", [BPC * N, H], f32, kind="ExternalInput").ap()
    encw_d = nc.dram_tensor("encw", [DIN, H], bf16, kind="ExternalInput").ap()
    # attention weights, natural [in, out] layout: [layer, (q,k,v,o), H, H]
    watt_d = nc.dram_tensor("watt", [H, 2, 4, H], bf16, kind="ExternalInput").ap()
    # per-partition biases: [128, 8]: col l*4+k for (q,k,v,o); encb [128,1]
    attb_d = nc.dram_tensor("attb", [H, 8], f32, kind="ExternalInput").ap()
    encb_d = nc.dram_tensor("encb", [H, 1], f32, kind="ExternalInput").ap()
    bv_d = nc.dram_tensor("bv", [2, H], f32, kind="ExternalInput").ap()
    wih_d = nc.dram_tensor("wih_t", [H, H3], f32, kind="ExternalInput").ap()
    whh_d = nc.dram_tensor("whh_t", [H, H3], f32, kind="ExternalInput").ap()
    # grub cols: 0: b_ih_r+b_hh_r, 1: b_ih_z+b_hh_z, 2: b_ih_n, 3: b_hh_n
    grub_d = nc.dram_tensor("grub", [H, 4], f32, kind="ExternalInput").ap()
    fcw_d = nc.dram_tensor("fcw", [H, A], f32, kind="ExternalInput").ap()
    fcb_d = nc.dram_tensor("fcb", [A], f32, kind="ExternalInput").ap()

    qout_d = nc.dram_tensor("qout", [BPC * N, A], f32, kind="ExternalOutput").ap()
    hout_d = nc.dram_tensor("hout", [BPC * N, H], f32, kind="ExternalOutput").ap()

    def bcast_ap(src):
        return bass.AP(
            tensor=src.tensor, offset=src.offset,
            ap=[[0, 128]] + [list(p) for p in src.ap],
        )

    with tile.TileContext(nc) as tc:
        with (
            tc.tile_pool(name="singles", bufs=1) as singles,
            tc.tile_pool(name="mn", bufs=4) as p_mn,          # mask natural f32
            tc.tile_pool(name="mb", bufs=6) as p_mb,          # mask natural bf16
            tc.tile_pool(name="mT", bufs=2) as p_mT,          # mask transposed bf16
            tc.tile_pool(name="xs", bufs=3) as p_xs,
            tc.tile_pool(name="hT", bufs=2) as p_hT,          # h1T/qT/kT/h2T/outT
            tc.tile_pool(name="v", bufs=NT + 2) as p_v,
            tc.tile_pool(name="e", bufs=3) as p_e,
            tc.tile_pool(name="p", bufs=3) as p_p,
            tc.tile_pool(name="h3", bufs=BPC) as p_h3,
            tc.tile_pool(name="r", bufs=2) as p_r,
            tc.tile_pool(name="gru", bufs=1) as p_gru,
            tc.tile_pool(name="out", bufs=4) as p_out,
            tc.tile_pool(name="pmm", bufs=2, space="PSUM") as pmm,
            tc.tile_pool(name="pacc", bufs=1, space="PSUM") as pacc,
            tc.tile_pool(name="prs", bufs=1, space="PSUM") as prs,
        ):
            masks = {}

            def emit_mask(b):
                # mask -> maskT (bf16) via 3D XBAR, one per row-tile
                mTa = p_mT.tile([128, NT, NP], bf16, tag="mT")
                for it in range(NT):
                    ts = _ts(it)
                    mn = p_mn.tile([128, NP], f32, tag="mn")
                    if ts < 128:
                        nc.vector.memset(mn, 0.0)
                    else:
                        nc.vector.memset(mn[:, N:NP], 0.0)
                    for ch in range(4):
                        r0, r1 = ch * 32, min((ch + 1) * 32, ts)
                        if r0 >= r1:
                            break
                        eng = nc.sync if ch % 2 == 0 else nc.scalar
                        eng.dma_start(
                            out=mn[r0:r1, :N],
                            in_=mask_d[b, it * 128 + r0 : it * 128 + r1, :],
                        )
                    mb = p_mb.tile([128, NP], bf16, tag="mb")
                    nc.vector.tensor_copy(out=mb, in_=mn)
                    eng = nc.sync if it % 2 == 0 else nc.scalar
                    eng.dma_start_transpose(
                        mTa[:, :, it * 128 : (it + 1) * 128], mb
                    )
                mT = [mTa[:, e, :] for e in range(NT)]
                # avoid 0-rowsum NaNs in padded query columns
                nc.vector.memset(mT[0][0:1, N:NP], 1.0)
                return mT

            masks[0] = emit_mask(0)

            # ---------- constants / weights ----------
            id_bf = singles.tile([128, 128], bf16)
            make_identity(nc, id_bf)
            id_f32 = singles.tile([128, 128], f32)
            make_identity(nc, id_f32)
            ones_bf = singles.tile([128, 128], bf16)
            nc.vector.memset(ones_bf, 1.0)

            encw_sb = singles.tile([DIN, H], bf16)
            nc.sync.dma_start(out=encw_sb, in_=encw_d)
            watt_sb = singles.tile([H, 2, 4, H], bf16)
            nc.sync.dma_start(out=watt_sb, in_=watt_d)
            attb_sb = singles.tile([H, 8], f32)
            nc.sync.dma_start(out=attb_sb, in_=attb_d)
            encb_sb = singles.tile([H, 1], f32)
            nc.sync.dma_start(out=encb_sb, in_=encb_d)
            bvb_sb = singles.tile([128, 2, H], f32)
            nc.sync.dma_start(out=bvb_sb, in_=bcast_ap(bv_d))
            wih_sb = singles.tile([H, H3], f32)
            nc.sync.dma_start(out=wih_sb, in_=wih_d)
            whh_sb = singles.tile([H, H3], f32)
            nc.sync.dma_start(out=whh_sb, in_=whh_d)
            wihr_sb = singles.tile([H, H3], f32r)
            nc.vector.tensor_copy(out=wihr_sb, in_=wih_sb)
            whhr_sb = singles.tile([H, H3], f32r)
            nc.vector.tensor_copy(out=whhr_sb, in_=whh_sb)
            grub_sb = singles.tile([H, 4], f32)
            nc.sync.dma_start(out=grub_sb, in_=grub_d)
            fcw_sb = singles.tile([H, A], f32)
            nc.sync.dma_start(out=fcw_sb, in_=fcw_d)
            fcb_sb = singles.tile([128, A], f32)
            nc.sync.dma_start(out=fcb_sb, in_=bcast_ap(fcb_d))

            h3Ts = {}

            # =================== attention (one batch) ===================
            def emit_attention(b):
                mT = masks.pop(b) if b in masks else emit_mask(b)

                # ---- x -> xT (bf16, padded cols zero) ----
                xT = p_xs.tile([DIN, NP], bf16, tag="xT")
                for it in range(NT):
                    ts = _ts(it)
                    xt = p_xs.tile([128, DIN], f32, tag="xt")
                    if ts < 128:
                        nc.vector.memset(xt, 0.0)
                    nc.sync.dma_start(
                        out=xt[:ts, :], in_=x_d[b, it * 128 : it * 128 + ts, :]
                    )
                    xb = p_xs.tile([128, DIN], bf16, tag="xb")
                    nc.vector.tensor_copy(out=xb, in_=xt)
                    pst = pmm.tile([DIN, NP], bf16, tag="mm")
                    nc.tensor.transpose(pst[:, :128], xb, id_bf)
                    nc.vector.tensor_copy(
                        out=xT[:, it * 128 : (it + 1) * 128], in_=pst[:, :128]
                    )

                # ---- h1T = relu(encW.T @ xT + encb) ----
                h1T = p_hT.tile([H, NP], bf16, tag="h1")
                ps = pmm.tile([128, NP], f32, tag="mm")
                for hf in range(2):
                    nc.tensor.matmul(
                        ps[:, hf * F : (hf + 1) * F], encw_sb,
                        xT[:, hf * F : (hf + 1) * F], start=True, stop=True,
                    )
                nc.scalar.activation(
                    out=h1T, in_=ps, func=AF.Relu, bias=encb_sb, scale=1.0
                )

                # ---- two attention layers ----
                hT = h1T
                for l in range(2):
                    qT = p_hT.tile([H, NP], bf16, tag="q")
                    kT = p_hT.tile([H, NP], bf16, tag="k")
                    for (dst, wi) in ((qT, 0), (kT, 1)):
                        ps = pmm.tile([128, NP], f32, tag="mm")
                        for hf in range(2):
                            nc.tensor.matmul(
                                ps[:, hf * F : (hf + 1) * F], watt_sb[:, l, wi, :],
                                hT[:, hf * F : (hf + 1) * F], start=True, stop=True,
                            )
                        nc.scalar.activation(
                            out=dst, in_=ps, func=AF.Relu,
                            bias=attb_sb[:, l * 4 + wi : l * 4 + wi + 1], scale=1.0,
                        )
                    # v in natural layout [node, H] (one tile per 128 nodes)
                    vs = []
                    for jt in range(NT):
                        ps = pmm.tile([128, NP], f32, tag="mm")
                        nc.tensor.matmul(
                            ps[:, :H], hT[:, jt * 128 : (jt + 1) * 128],
                            watt_sb[:, l, 2, :], start=True, stop=True,
                        )
                        vt = p_v.tile([128, H], f32, tag="vf")
                        nc.vector.scalar_tensor_tensor(
                            out=vt, in0=ps[:, :H], scalar=0.0,
                            in1=bvb_sb[:, l, :], op0=OP.bypass, op1=OP.add,
                        )
                        vb = p_v.tile([128, H], bf16, tag="vb")
                        nc.vector.tensor_scalar_max(vb, vt, 0.0)
                        vs.append(vb)

                    psA = pacc.tile([128, NP], f32, tag="acc")
                    psR = prs.tile([128, NP], f32, tag="rs")
                    for jt in range(NT):
                        psS = pmm.tile([128, NP], f32, tag="mm")
                        for hf in range(2):
                            nc.tensor.matmul(
                                psS[:, hf * F : (hf + 1) * F],
                                kT[:, jt * 128 : (jt + 1) * 128],
                                qT[:, hf * F : (hf + 1) * F],
                                start=True, stop=True,
                            )
                        et = p_e.tile([128, NP], bf16, tag="e")
                        nc.scalar.activation(
                            out=et, in_=psS, func=AF.Exp, bias=0.0, scale=1.0
                        )
                        pt = p_p.tile([128, NP], bf16, tag="p")
                        nc.vector.tensor_mul(pt, et, mT[jt])
                        for hf in range(2):
                            nc.tensor.matmul(
                                psA[:, hf * F : (hf + 1) * F], vs[jt],
                                pt[:, hf * F : (hf + 1) * F],
                                start=(jt == 0), stop=(jt == NT - 1),
                            )
                            nc.tensor.matmul(
                                psR[:, hf * F : (hf + 1) * F], ones_bf,
                                pt[:, hf * F : (hf + 1) * F],
                                start=(jt == 0), stop=(jt == NT - 1),
                            )
                    r_all = p_r.tile([128, NP], f32, tag="r")
                    r_scr = p_r.tile([128, NP], f32, tag="rscr", bufs=1)
                    outT = p_hT.tile([H, NP], bf16, tag="o")
                    nc.vector.reciprocal_approx_accurate(
                        out=r_all, in_=psR, scratch=r_scr
                    )
                    for hf in range(2):
                        sl = slice(hf * F, (hf + 1) * F)
                        nc.vector.tensor_mul(outT[:, sl], psA[:, sl], r_all[:, sl])
                    # h_next = relu(Wo.T @ outT + bo)
                    if l == 0:
                        hnext = p_hT.tile([H, NP], bf16, tag="h2")
                    else:
                        hnext = p_h3.tile([H, NP], f32r, tag="h3")
                    ps = pmm.tile([128, NP], f32, tag="mm")
                    for hf in range(2):
                        nc.tensor.matmul(
                            ps[:, hf * F : (hf + 1) * F], watt_sb[:, l, 3, :],
                            outT[:, hf * F : (hf + 1) * F], start=True, stop=True,
                        )
                    nc.scalar.activation(
                        out=hnext, in_=ps, func=AF.Relu,
                        bias=attb_sb[:, l * 4 + 3 : l * 4 + 4], scale=1.0,
                    )
                    hT = hnext
                h3Ts[b] = hT

            # =================== GRU (one batch, float32r matmuls) ===========
            def emit_hid_pipe(b):
                hinT_t = p_gru.tile([H, NP], f32, tag="hin", bufs=2)
                for it in range(NT):
                    ts = _ts(it)
                    ht_f = p_xs.tile([128, H], f32, tag="hidt")
                    if ts < 128:
                        nc.vector.memset(ht_f, 0.0)
                    nc.sync.dma_start(
                        out=ht_f[:ts, :],
                        in_=hid_d[b * N + it * 128 : b * N + it * 128 + ts, :],
                    )
                    pst = pmm.tile([128, NP], f32, tag="mm")
                    nc.tensor.transpose(pst[:, :128], ht_f, id_f32)
                    nc.vector.tensor_copy(
                        out=hinT_t[:, it * 128 : (it + 1) * 128], in_=pst[:, :128]
                    )
                hinT = p_gru.tile([H, NP], f32r, tag="hinr", bufs=2)
                nc.vector.tensor_copy(out=hinT, in_=hinT_t)
                return hinT_t, hinT

            def emit_gru(b, hin_pair):
                h3T = h3Ts[b]
                hinT_t, hinT = hin_pair

                # r and z gates: sigmoid(W_ih_g.T@h3T + W_hh_g.T@hinT + b_g)
                gates = []
                for g in range(2):
                    gt = p_gru.tile([H, NP], f32, tag=f"g{g}", bufs=2)
                    ps = pmm.tile([128, NP], f32, tag="mm")
                    for hf in range(2):
                        sl = slice(hf * F, (hf + 1) * F)
                        nc.tensor.matmul(
                            ps[:, sl], wihr_sb[:, g * H : (g + 1) * H],
                            h3T[:, sl], start=True, stop=False,
                        )
                        nc.tensor.matmul(
                            ps[:, sl], whhr_sb[:, g * H : (g + 1) * H],
                            hinT[:, sl], start=False, stop=True,
                        )
                    nc.scalar.activation(
                        out=gt, in_=ps, func=AF.Sigmoid,
                        bias=grub_sb[:, g : g + 1], scale=1.0,
                    )
                    gates.append(gt)
                rg, zg = gates
                # n = tanh(xn + b_ihn + r*(hn + b_hhn))
                ng = p_gru.tile([H, NP], f32, tag="n", bufs=2)
                psX = pmm.tile([128, NP], f32, tag="mm")
                psHn = pmm.tile([128, NP], f32, tag="mm")
                for hf in range(2):
                    sl = slice(hf * F, (hf + 1) * F)
                    nc.tensor.matmul(
                        psX[:, sl], wihr_sb[:, 2 * H : 3 * H],
                        h3T[:, sl], start=True, stop=True,
                    )
                    nc.tensor.matmul(
                        psHn[:, sl], whhr_sb[:, 2 * H : 3 * H],
                        hinT[:, sl], start=True, stop=True,
                    )
                tt = p_gru.tile([H, NP], f32, tag="t")
                pre = p_gru.tile([H, NP], f32, tag="pre")
                for hf in range(2):
                    sl = slice(hf * F, (hf + 1) * F)
                    nc.vector.scalar_tensor_tensor(
                        out=tt[:, sl], in0=psHn[:, sl], scalar=grub_sb[:, 3:4],
                        in1=rg[:, sl], op0=OP.add, op1=OP.mult,
                    )
                    nc.vector.scalar_tensor_tensor(
                        out=pre[:, sl], in0=psX[:, sl], scalar=grub_sb[:, 2:3],
                        in1=tt[:, sl], op0=OP.add, op1=OP.add,
                    )
                nc.scalar.activation(
                    out=ng, in_=pre, func=AF.Tanh, bias=0.0, scale=1.0
                )
                # h = n + z*(hin - n)
                dd = p_gru.tile([H, NP], f32, tag="d")
                nc.vector.tensor_sub(dd, hinT_t, ng)
                nc.vector.tensor_mul(dd, dd, zg)
                hnT = p_gru.tile([H, NP], f32, tag="hn")
                nc.vector.tensor_add(hnT, dd, ng)

                # outputs
                for it in range(NT):
                    ts = _ts(it)
                    sl = slice(it * 128, it * 128 + 128)
                    psQ = pmm.tile([128, NP], f32, tag="mm")
                    nc.tensor.matmul(
                        psQ[:, :A], hnT[:, sl],
                        fcw_sb, start=True, stop=True,
                    )
                    qo = p_out.tile([128, A], f32, tag="qo")
                    nc.vector.tensor_add(qo, psQ[:, :A], fcb_sb)
                    nc.sync.dma_start(
                        out=qout_d[b * N + it * 128 : b * N + it * 128 + ts, :],
                        in_=qo[:ts, :],
                    )
                    psT = pmm.tile([128, NP], f32, tag="mm")
                    nc.tensor.transpose(psT[:, :128], hnT[:, sl], id_f32)
                    hsb = p_out.tile([128, H], f32, tag="ho")
                    nc.vector.tensor_copy(out=hsb, in_=psT[:, :128])
                    nc.sync.dma_start(
                        out=hout_d[b * N + it * 128 : b * N + it * 128 + ts, :],
                        in_=hsb[:ts, :],
                    )

            # ============ driver: attention phase then GRU phase ============
            for b in range(BPC):
                emit_attention(b)
            hins = {0: emit_hid_pipe(0)}
            for b in range(BPC):
                if b + 1 < BPC:
                    hins[b + 1] = emit_hid_pipe(b + 1)
                emit_gru(b, hins.pop(b))

    nc.compile()
    return nc


@functools.lru_cache(maxsize=1)
def _get_program():
    return build_program()


def _prep_weights(inputs):
    bf = ml_dtypes.bfloat16
    f = np.float32
    out = {}
    out["encw"] = np.ascontiguousarray(inputs["enc_W"], dtype=bf)
    watt = np.stack(
        [
            np.stack([inputs["Wq1"], inputs["Wk1"], inputs["Wv1"], inputs["Wo1"]]),
            np.stack([inputs["Wq2"], inputs["Wk2"], inputs["Wv2"], inputs["Wo2"]]),
        ]
    )
    out["watt"] = np.ascontiguousarray(watt.transpose(2, 0, 1, 3), dtype=bf)
    attb = np.stack(
        [
            inputs["bq1"], inputs["bk1"], inputs["bv1"], inputs["bo1"],
            inputs["bq2"], inputs["bk2"], inputs["bv2"], inputs["bo2"],
        ],
        axis=1,
    )  # [128, 8]
    out["attb"] = np.ascontiguousarray(attb, dtype=f)
    out["encb"] = np.ascontiguousarray(inputs["enc_b"][:, None], dtype=f)
    out["bv"] = np.ascontiguousarray(
        np.stack([inputs["bv1"], inputs["bv2"]]), dtype=f
    )
    out["wih_t"] = np.ascontiguousarray(inputs["W_ih"].T, dtype=f)
    out["whh_t"] = np.ascontiguousarray(inputs["W_hh"].T, dtype=f)
    b_ih, b_hh = inputs["b_ih"], inputs["b_hh"]
    grub = np.stack(
        [
            b_ih[0:H] + b_hh[0:H],
            b_ih[H : 2 * H] + b_hh[H : 2 * H],
            b_ih[2 * H : 3 * H],
            b_hh[2 * H : 3 * H],
        ],
        axis=1,
    )  # [128, 4]
    out["grub"] = np.ascontiguousarray(grub, dtype=f)
    out["fcw"] = np.ascontiguousarray(inputs["fc_W"], dtype=f)
    out["fcb"] = np.ascontiguousarray(inputs["fc_b"], dtype=f)
    return out


def run(inputs, trace=False):
    nc = _get_program()
    w = _prep_weights(inputs)
    x = np.asarray(inputs["x"], dtype=np.float32)
    mask = np.asarray(inputs["mask"], dtype=np.float32)
    hid = np.asarray(inputs["hidden_state"], dtype=np.float32)
    in_maps = []
    for c in range(NCORES):
        bs = slice(c * BPC, (c + 1) * BPC)
        m = dict(w)
        m["x"] = np.ascontiguousarray(x[bs])
        m["mask"] = np.ascontiguousarray(mask[bs])
        m["hid"] = np.ascontiguousarray(hid[c * BPC * N : (c + 1) * BPC * N])
        in_maps.append(m)
    res = bass_utils.run_bass_kernel_spmd(
        nc, in_maps, core_ids=list(range(NCORES)), trace=trace
    )
    q_out = np.concatenate([r["qout"] for r in res.results], axis=0)
    h_out = np.concatenate([r["hout"] for r in res.results], axis=0)
    return (q_out, h_out), res


def kernel(**inputs):
    (q_out, h_out), _ = run(inputs)
    return q_out, h_out


# revision 15
# speedup vs baseline: 1.0206x; 1.0206x over previous
"""Trainium2 Bass kernel for nn_DGN (graph attention network + GRU).

Data-parallel over batch: 32 batches -> 8 cores x 4 batches.
Layout strategy: all per-node activations kept transposed [H=128, N] so that
H (=128) sits on partitions. Scores are computed transposed ([keys, queries])
so the attention-weight matrix is directly usable as the moving operand of the
att@v matmul (contraction over keys on partitions) -- no per-layer transposes.
Softmax has no row-max subtraction (scores empirically in [0, ~5]); the row
sums come from a ones-weights matmul that also broadcasts them to all
partitions for free.  The mask is transposed per batch via 3D XBAR DMA in
bf16 (out[p, e, f] = in[f, e*128+p]).  GRU matmuls run in float32r.
"""

import os
import sys
import functools
import numpy as np
import ml_dtypes

sys.path.insert(0, "/opt/trn_rl_repo")

import concourse.bass as bass
import concourse.bacc as bacc
import concourse.tile as tile
import concourse.mybir as mybir
from concourse import bass_utils
from concourse.masks import make_identity

B, N, DIN, H, A = 32, 1000, 64, 128, 10
NCORES = 8
BPC = B // NCORES  # batches per core
NP = 1024          # padded node count
NT = 8             # 128-row tiles per batch
F = 512            # psum half (one bank of fp32)
H3 = 3 * H

f32 = mybir.dt.float32
f32r = mybir.dt.float32r
bf16 = mybir.dt.bfloat16
AF = mybir.ActivationFunctionType
OP = mybir.AluOpType

def _ts(it):
    """rows of tile it that hold real data (last tile ragged: 1000 = 7*128+104)"""
    return N - it * 128 if it == NT - 1 else 128


def build_program():
    nc = bacc.Bacc("TRN2", debug=False, num_devices=NCORES)

    # ---- DRAM I/O ----
    x_d = nc.dram_tensor("x", [BPC, N, DIN], f32, kind="ExternalInput").ap()
    mask_d = nc.dram_tensor("mask", [BPC, N, N], f32, kind="ExternalInput").ap()
    hid_d = nc.dram_tensor("hid", [BPC * N, H], f32, kind="ExternalInput").ap()
    encw_d = nc.dram_tensor("encw", [DIN, H], bf16, kind="ExternalInput").ap()
    # attention weights, natural [in, out] layout: [layer, (q,k,v,o), H, H]
    watt_d = nc.dram_tensor("watt", [H, 2, 4, H], bf16, kind="ExternalInput").ap()
    # per-partition biases: [128, 8]: col l*4+k for (q,k,v,o); encb [128,1]
    attb_d = nc.dram_tensor("attb", [H, 8], f32, kind="ExternalInput").ap()
    encb_d = nc.dram_tensor("encb", [H, 1], f32, kind="ExternalInput").ap()
    bv_d = nc.dram_tensor("bv", [2, H], f32, kind="ExternalInput").ap()
    wih_d = nc.dram_tensor("wih_t", [H, H3], f32, kind="ExternalInput").ap()
    whh_d = nc.dram_tensor("whh_t", [H, H3], f32, kind="ExternalInput").ap()
    # grub cols: 0: b_ih_r+b_hh_r, 1: b_ih_z+b_hh_z, 2: b_ih_n, 3: b_hh_n
    grub_d = nc.dram_tensor("grub", [H, 4], f32, kind="ExternalInput").ap()
    fcw_d = nc.dram_tensor("fcw", [H, A], f32, kind="ExternalInput").ap()
    fcb_d = nc.dram_tensor("fcb", [A], f32, kind="ExternalInput").ap()

    qout_d = nc.dram_tensor("qout", [BPC * N, A], f32, kind="ExternalOutput").ap()
    hout_d = nc.dram_tensor("hout", [BPC * N, H], f32, kind="ExternalOutput").ap()

    def bcast_ap(src):
        return bass.AP(
            tensor=src.tensor, offset=src.offset,
            ap=[[0, 128]] + [list(p) for p in src.ap],
        )

    with tile.TileContext(nc) as tc:
        with (
            tc.tile_pool(name="singles", bufs=1) as singles,
            tc.tile_pool(name="mn", bufs=4) as p_mn,          # mask natural f32
            tc.tile_pool(name="mb", bufs=6) as p_mb,          # mask natural bf16
            tc.tile_pool(name="mT", bufs=2) as p_mT,          # mask transposed bf16
            tc.tile_pool(name="xs", bufs=3) as p_xs,
            tc.tile_pool(name="hT", bufs=2) as p_hT,          # h1T/qT/kT/h2T/outT
            tc.tile_pool(name="v", bufs=NT + 2) as p_v,
            tc.tile_pool(name="e", bufs=3) as p_e,
            tc.tile_pool(name="p", bufs=3) as p_p,
            tc.tile_pool(name="h3", bufs=BPC) as p_h3,
            tc.tile_pool(name="r", bufs=2) as p_r,
            tc.tile_pool(name="gru", bufs=1) as p_gru,
            tc.tile_pool(name="out", bufs=4) as p_out,
            tc.tile_pool(name="pmm", bufs=2, space="PSUM") as pmm,
            tc.tile_pool(name="pacc", bufs=1, space="PSUM") as pacc,
            tc.tile_pool(name="prs", bufs=1, space="PSUM") as prs,
        ):
            masks = {}

            def emit_mask(b):
                # mask -> maskT (bf16) via 3D XBAR, one per row-tile
                mTa = p_mT.tile([128, NT, NP], bf16, tag="mT")
                for it in range(NT):
                    ts = _ts(it)
                    mn = p_mn.tile([128, NP], f32, tag="mn")
                    if ts < 128:
                        nc.vector.memset(mn, 0.0)
                    else:
                        nc.vector.memset(mn[:, N:NP], 0.0)
                    for ch in range(4):
                        r0, r1 = ch * 32, min((ch + 1) * 32, ts)
                        if r0 >= r1:
                            break
                        eng = nc.sync if ch % 2 == 0 else nc.scalar
                        eng.dma_start(
                            out=mn[r0:r1, :N],
                            in_=mask_d[b, it * 128 + r0 : it * 128 + r1, :],
                        )
                    mb = p_mb.tile([128, NP], bf16, tag="mb")
                    nc.vector.tensor_copy(out=mb, in_=mn)
                    eng = nc.sync if it % 2 == 0 else nc.scalar
                    eng.dma_start_transpose(
                        mTa[:, :, it * 128 : (it + 1) * 128], mb
                    )
                mT = [mTa[:, e, :] for e in range(NT)]
                # avoid 0-rowsum NaNs in padded query columns
                nc.vector.memset(mT[0][0:1, N:NP], 1.0)
                return mT

            # ---------- constants / weights ----------
            id_bf = singles.tile([128, 128], bf16)
            make_identity(nc, id_bf)
            id_f32 = singles.tile([128, 128], f32)
            make_identity(nc, id_f32)
            ones_bf = singles.tile([128, 128], bf16)
            nc.vector.memset(ones_bf, 1.0)

            encw_sb = singles.tile([DIN, H], bf16)
            nc.sync.dma_start(out=encw_sb, in_=encw_d)
            watt_sb = singles.tile([H, 2, 4, H], bf16)
            nc.sync.dma_start(out=watt_sb, in_=watt_d)
            attb_sb = singles.tile([H, 8], f32)
            nc.sync.dma_start(out=attb_sb, in_=attb_d)
            encb_sb = singles.tile([H, 1], f32)
            nc.sync.dma_start(out=encb_sb, in_=encb_d)
            bvb_sb = singles.tile([128, 2, H], f32)
            nc.sync.dma_start(out=bvb_sb, in_=bcast_ap(bv_d))
            wih_sb = singles.tile([H, H3], f32)
            nc.sync.dma_start(out=wih_sb, in_=wih_d)
            whh_sb = singles.tile([H, H3], f32)
            nc.sync.dma_start(out=whh_sb, in_=whh_d)
            wihr_sb = singles.tile([H, H3], f32r)
            nc.vector.tensor_copy(out=wihr_sb, in_=wih_sb)
            whhr_sb = singles.tile([H, H3], f32r)
            nc.vector.tensor_copy(out=whhr_sb, in_=whh_sb)
            grub_sb = singles.tile([H, 4], f32)
            nc.sync.dma_start(out=grub_sb, in_=grub_d)
            fcw_sb = singles.tile([H, A], f32)
            nc.sync.dma_start(out=fcw_sb, in_=fcw_d)
            fcb_sb = singles.tile([128, A], f32)
            nc.sync.dma_start(out=fcb_sb, in_=bcast_ap(fcb_d))

            h3Ts = {}

            # =================== attention (one batch) ===================
            def emit_attention(b):
                mT = masks.pop(b) if b in masks else emit_mask(b)

                # ---- x -> xT (bf16, padded cols zero) ----
                xT = p_xs.tile([DIN, NP], bf16, tag="xT")
                for it in range(NT):
                    ts = _ts(it)
                    xt = p_xs.tile([128, DIN], f32, tag="xt")
                    if ts < 128:
                        nc.vector.memset(xt, 0.0)
                    nc.sync.dma_start(
                        out=xt[:ts, :], in_=x_d[b, it * 128 : it * 128 + ts, :]
                    )
                    xb = p_xs.tile([128, DIN], bf16, tag="xb")
                    nc.vector.tensor_copy(out=xb, in_=xt)
                    pst = pmm.tile([DIN, NP], bf16, tag="mm")
                    nc.tensor.transpose(pst[:, :128], xb, id_bf)
                    nc.vector.tensor_copy(
                        out=xT[:, it * 128 : (it + 1) * 128], in_=pst[:, :128]
                    )

                # ---- h1T = relu(encW.T @ xT + encb) ----
                h1T = p_hT.tile([H, NP], bf16, tag="h1")
                ps = pmm.tile([128, NP], f32, tag="mm")
                for hf in range(2):
                    nc.tensor.matmul(
                        ps[:, hf * F : (hf + 1) * F], encw_sb,
                        xT[:, hf * F : (hf + 1) * F], start=True, stop=True,
                    )
                nc.scalar.activation(
                    out=h1T, in_=ps, func=AF.Relu, bias=encb_sb, scale=1.0
                )

                # ---- two attention layers ----
                hT = h1T
                for l in range(2):
                    qT = p_hT.tile([H, NP], bf16, tag="q")
                    kT = p_hT.tile([H, NP], bf16, tag="k")
                    for (dst, wi) in ((qT, 0), (kT, 1)):
                        ps = pmm.tile([128, NP], f32, tag="mm")
                        for hf in range(2):
                            nc.tensor.matmul(
                                ps[:, hf * F : (hf + 1) * F], watt_sb[:, l, wi, :],
                                hT[:, hf * F : (hf + 1) * F], start=True, stop=True,
                            )
                        nc.scalar.activation(
                            out=dst, in_=ps, func=AF.Relu,
                            bias=attb_sb[:, l * 4 + wi : l * 4 + wi + 1], scale=1.0,
                        )
                    # v in natural layout [node, H] (one tile per 128 nodes)
                    vs = []
                    for jt in range(NT):
                        ps = pmm.tile([128, NP], f32, tag="mm")
                        nc.tensor.matmul(
                            ps[:, :H], hT[:, jt * 128 : (jt + 1) * 128],
                            watt_sb[:, l, 2, :], start=True, stop=True,
                        )
                        vt = p_v.tile([128, H], f32, tag="vf")
                        nc.vector.scalar_tensor_tensor(
                            out=vt, in0=ps[:, :H], scalar=0.0,
                            in1=bvb_sb[:, l, :], op0=OP.bypass, op1=OP.add,
                        )
                        vb = p_v.tile([128, H], bf16, tag="vb")
                        nc.vector.tensor_scalar_max(vb, vt, 0.0)
                        vs.append(vb)

                    psA = pacc.tile([128, NP], f32, tag="acc")
                    psR = prs.tile([128, NP], f32, tag="rs")
                    for jt in range(NT):
                        psS = pmm.tile([128, NP], f32, tag="mm")
                        for hf in range(2):
                            nc.tensor.matmul(
                                psS[:, hf * F : (hf + 1) * F],
                                kT[:, jt * 128 : (jt + 1) * 128],
                                qT[:, hf * F : (hf + 1) * F],
                                start=True, stop=True,
                            )
                        et = p_e.tile([128, NP], bf16, tag="e")
                        nc.scalar.activation(
                            out=et, in_=psS, func=AF.Exp, bias=0.0, scale=1.0
                        )
                        pt = p_p.tile([128, NP], bf16, tag="p")
                        nc.vector.tensor_mul(pt, et, mT[jt])
                        for hf in range(2):
                            nc.tensor.matmul(
                                psA[:, hf * F : (hf + 1) * F], vs[jt],
                                pt[:, hf * F : (hf + 1) * F],
                                start=(jt == 0), stop=(jt == NT - 1),
                            )
                            nc.tensor.matmul(
                                psR[:, hf * F : (hf + 1) * F], ones_bf,
                                pt[:, hf * F : (hf + 1) * F],
                                start=(jt == 0), stop=(jt == NT - 1),
                            )
                    r_all = p_r.tile([128, NP], f32, tag="r")
                    r_scr = p_r.tile([128, NP], f32, tag="rscr", bufs=1)
                    outT = p_hT.tile([H, NP], bf16, tag="o")
                    nc.vector.reciprocal_approx_accurate(
                        out=r_all, in_=psR, scratch=r_scr
                    )
                    for hf in range(2):
                        sl = slice(hf * F, (hf + 1) * F)
                        nc.vector.tensor_mul(outT[:, sl], psA[:, sl], r_all[:, sl])
                    # h_next = relu(Wo.T @ outT + bo)
                    if l == 0:
                        hnext = p_hT.tile([H, NP], bf16, tag="h2")
                    else:
                        hnext = p_h3.tile([H, NP], f32r, tag="h3")
                    ps = pmm.tile([128, NP], f32, tag="mm")
                    for hf in range(2):
                        nc.tensor.matmul(
                            ps[:, hf * F : (hf + 1) * F], watt_sb[:, l, 3, :],
                            outT[:, hf * F : (hf + 1) * F], start=True, stop=True,
                        )
                    nc.scalar.activation(
                        out=hnext, in_=ps, func=AF.Relu,
                        bias=attb_sb[:, l * 4 + 3 : l * 4 + 4], scale=1.0,
                    )
                    hT = hnext
                h3Ts[b] = hT

            # =================== GRU (one batch, float32r matmuls) ===========
            def emit_hid_pipe(b):
                hinT_t = p_gru.tile([H, NP], f32, tag="hin", bufs=2)
                for it in range(NT):
                    ts = _ts(it)
                    ht_f = p_xs.tile([128, H], f32, tag="hidt")
                    if ts < 128:
                        nc.vector.memset(ht_f, 0.0)
                    nc.sync.dma_start(
                        out=ht_f[:ts, :],
                        in_=hid_d[b * N + it * 128 : b * N + it * 128 + ts, :],
                    )
                    pst = pmm.tile([128, NP], f32, tag="mm")
                    nc.tensor.transpose(pst[:, :128], ht_f, id_f32)
                    nc.vector.tensor_copy(
                        out=hinT_t[:, it * 128 : (it + 1) * 128], in_=pst[:, :128]
                    )
                hinT = p_gru.tile([H, NP], f32r, tag="hinr", bufs=2)
                nc.vector.tensor_copy(out=hinT, in_=hinT_t)
                return hinT_t, hinT

            def emit_gru(b, hin_pair):
                h3T = h3Ts[b]
                hinT_t, hinT = hin_pair

                # r and z gates: sigmoid(W_ih_g.T@h3T + W_hh_g.T@hinT + b_g)
                gates = []
                for g in range(2):
                    gt = p_gru.tile([H, NP], f32, tag=f"g{g}", bufs=2)
                    ps = pmm.tile([128, NP], f32, tag="mm")
                    for hf in range(2):
                        sl = slice(hf * F, (hf + 1) * F)
                        nc.tensor.matmul(
                            ps[:, sl], wihr_sb[:, g * H : (g + 1) * H],
                            h3T[:, sl], start=True, stop=False,
                        )
                        nc.tensor.matmul(
                            ps[:, sl], whhr_sb[:, g * H : (g + 1) * H],
                            hinT[:, sl], start=False, stop=True,
                        )
                    nc.scalar.activation(
                        out=gt, in_=ps, func=AF.Sigmoid,
                        bias=grub_sb[:, g : g + 1], scale=1.0,
                    )
                    gates.append(gt)
                rg, zg = gates
                # n = tanh(xn + b_ihn + r*(hn + b_hhn))
                ng = p_gru.tile([H, NP], f32, tag="n", bufs=2)
                psX = pmm.tile([128, NP], f32, tag="mm")
                psHn = pmm.tile([128, NP], f32, tag="mm")
                for hf in range(2):
                    sl = slice(hf * F, (hf + 1) * F)
                    nc.tensor.matmul(
                        psX[:, sl], wihr_sb[:, 2 * H : 3 * H],
                        h3T[:, sl], start=True, stop=True,
                    )
                    nc.tensor.matmul(
                        psHn[:, sl], whhr_sb[:, 2 * H : 3 * H],
                        hinT[:, sl], start=True, stop=True,
                    )
                tt = p_gru.tile([H, NP], f32, tag="t")
                pre = p_gru.tile([H, NP], f32, tag="pre")
                for hf in range(2):
                    sl = slice(hf * F, (hf + 1) * F)
                    nc.vector.scalar_tensor_tensor(
                        out=tt[:, sl], in0=psHn[:, sl], scalar=grub_sb[:, 3:4],
                        in1=rg[:, sl], op0=OP.add, op1=OP.mult,
                    )
                    nc.vector.scalar_tensor_tensor(
                        out=pre[:, sl], in0=psX[:, sl], scalar=grub_sb[:, 2:3],
                        in1=tt[:, sl], op0=OP.add, op1=OP.add,
                    )
                nc.scalar.activation(
                    out=ng, in_=pre, func=AF.Tanh, bias=0.0, scale=1.0
                )
                # h = n + z*(hin - n)
                dd = p_gru.tile([H, NP], f32, tag="d")
                nc.vector.tensor_sub(dd, hinT_t, ng)
                nc.vector.tensor_mul(dd, dd, zg)
                hnT = p_gru.tile([H, NP], f32, tag="hn")
                nc.vector.tensor_add(hnT, dd, ng)

                # outputs
                for it in range(NT):
                    ts = _ts(it)
                    sl = slice(it * 128, it * 128 + 128)
                    psQ = pmm.tile([128, NP], f32, tag="mm")
                    nc.tensor.matmul(
                        psQ[:, :A], hnT[:, sl],
                        fcw_sb, start=True, stop=True,
                    )
                    qo = p_out.tile([128, A], f32, tag="qo")
                    nc.vector.tensor_add(qo, psQ[:, :A], fcb_sb)
                    nc.sync.dma_start(
                        out=qout_d[b * N + it * 128 : b * N + it * 128 + ts, :],
                        in_=qo[:ts, :],
                    )
                    psT = pmm.tile([128, NP], f32, tag="mm")
                    nc.tensor.transpose(psT[:, :128], hnT[:, sl], id_f32)
                    hsb = p_out.tile([128, H], f32, tag="ho")
                    nc.vector.tensor_copy(out=hsb, in_=psT[:, :128])
                    nc.sync.dma_start(
                        out=hout_d[b * N + it * 128 : b * N + it * 128 + ts, :],
                        in_=hsb[:ts, :],
                    )

            # ============ driver: attention phase then GRU phase ============
            for b in range(BPC):
                emit_attention(b)
            hins = {0: emit_hid_pipe(0)}
            for b in range(BPC):
                if b + 1 < BPC:
                    hins[b + 1] = emit_hid_pipe(b + 1)
                emit_gru(b, hins.pop(b))

    nc.compile()
    return nc


@functools.lru_cache(maxsize=1)
def _get_program():
    return build_program()


def _prep_weights(inputs):
    bf = ml_dtypes.bfloat16
    f = np.float32
    out = {}
    out["encw"] = np.ascontiguousarray(inputs["enc_W"], dtype=bf)
    watt = np.stack(
        [
            np.stack([inputs["Wq1"], inputs["Wk1"], inputs["Wv1"], inputs["Wo1"]]),
            np.stack([inputs["Wq2"], inputs["Wk2"], inputs["Wv2"], inputs["Wo2"]]),
        ]
    )
    out["watt"] = np.ascontiguousarray(watt.transpose(2, 0, 1, 3), dtype=bf)
    attb = np.stack(
        [
            inputs["bq1"], inputs["bk1"], inputs["bv1"], inputs["bo1"],
            inputs["bq2"], inputs["bk2"], inputs["bv2"], inputs["bo2"],
        ],
        axis=1,
    )  # [128, 8]
    out["attb"] = np.ascontiguousarray(attb, dtype=f)
    out["encb"] = np.ascontiguousarray(inputs["enc_b"][:, None], dtype=f)
    out["bv"] = np.ascontiguousarray(
        np.stack([inputs["bv1"], inputs["bv2"]]), dtype=f
    )
    out["wih_t"] = np.ascontiguousarray(inputs["W_ih"].T, dtype=f)
    out["whh_t"] = np.ascontiguousarray(inputs["W_hh"].T, dtype=f)
    b_ih, b_hh = inputs["b_ih"], inputs["b_hh"]
    grub = np.stack(
        [
            b_ih[0:H] + b_hh[0:H],
            b_ih[H : 2 * H] + b_hh[H : 2 * H],
            b_ih[2 * H : 3 * H],
            b_hh[2 * H : 3 * H],
        ],
        axis=1,
    )  # [128, 4]
    out["grub"] = np.ascontiguousarray(grub, dtype=f)
    out["fcw"] = np.ascontiguousarray(inputs["fc_W"], dtype=f)
    out["fcb"] = np.ascontiguousarray(inputs["fc_b"], dtype=f)
    return out


def run(inputs, trace=False):
    nc = _get_program()
    w = _prep_weights(inputs)
    x = np.asarray(inputs["x"], dtype=np.float32)
    mask = np.asarray(inputs["mask"], dtype=np.float32)
    hid = np.asarray(inputs["hidden_state"], dtype=np.float32)
    in_maps = []
    for c in range(NCORES):
        bs = slice(c * BPC, (c + 1) * BPC)
        m = dict(w)
        m["x"] = np.ascontiguousarray(x[bs])
        m["mask"] = np.ascontiguousarray(mask[bs])
        m["hid"] = np.ascontiguousarray(hid[c * BPC * N : (c + 1) * BPC * N])
        in_maps.append(m)
    res = bass_utils.run_bass_kernel_spmd(
        nc, in_maps, core_ids=list(range(NCORES)), trace=trace
    )
    q_out = np.concatenate([r["qout"] for r in res.results], axis=0)
    h_out = np.concatenate([r["hout"] for r in res.results], axis=0)
    return (q_out, h_out), res


def kernel(**inputs):
    (q_out, h_out), _ = run(inputs)
    return q_out, h_out


# revision 16
# speedup vs baseline: 1.0651x; 1.0436x over previous
"""Trainium2 Bass kernel for nn_DGN (graph attention network + GRU).

Data-parallel over batch: 32 batches -> 8 cores x 4 batches.
Layout strategy: all per-node activations kept transposed [H=128, N] so that
H (=128) sits on partitions. Scores are computed transposed ([keys, queries])
so the attention-weight matrix is directly usable as the moving operand of the
att@v matmul (contraction over keys on partitions) -- no per-layer transposes.
Softmax has no row-max subtraction (scores empirically in [0, ~5]); the row
sums come from a ones-weights matmul that also broadcasts them to all
partitions for free.  The mask is transposed per batch via 3D XBAR DMA in
bf16 (out[p, e, f] = in[f, e*128+p]).  GRU matmuls run in float32r.
"""

import os
import sys
import functools
import numpy as np
import ml_dtypes

sys.path.insert(0, "/opt/trn_rl_repo")

import concourse.bass as bass
import concourse.bacc as bacc
import concourse.tile as tile
import concourse.mybir as mybir
from concourse import bass_utils
from concourse.masks import make_identity

B, N, DIN, H, A = 32, 1000, 64, 128, 10
NCORES = 8
BPC = B // NCORES  # batches per core
NP = 1024          # padded node count
NT = 8             # 128-row tiles per batch
F = 512            # psum half (one bank of fp32)
H3 = 3 * H

f32 = mybir.dt.float32
f32r = mybir.dt.float32r
bf16 = mybir.dt.bfloat16
AF = mybir.ActivationFunctionType
OP = mybir.AluOpType

def _ts(it):
    """rows of tile it that hold real data (last tile ragged: 1000 = 7*128+104)"""
    return N - it * 128 if it == NT - 1 else 128


def build_program():
    nc = bacc.Bacc("TRN2", debug=False, num_devices=NCORES)

    # ---- DRAM I/O ----
    x_d = nc.dram_tensor("x", [BPC, N, DIN], f32, kind="ExternalInput").ap()
    mask_d = nc.dram_tensor("mask", [BPC, N, N], f32, kind="ExternalInput").ap()
    hid_d = nc.dram_tensor("hid", [BPC * N, H], f32, kind="ExternalInput").ap()
    encw_d = nc.dram_tensor("encw", [DIN, H], bf16, kind="ExternalInput").ap()
    # attention weights, natural [in, out] layout: [layer, (q,k,v,o), H, H]
    watt_d = nc.dram_tensor("watt", [H, 2, 4, H], bf16, kind="ExternalInput").ap()
    # per-partition biases: [128, 8]: col l*4+k for (q,k,v,o); encb [128,1]
    attb_d = nc.dram_tensor("attb", [H, 8], f32, kind="ExternalInput").ap()
    encb_d = nc.dram_tensor("encb", [H, 1], f32, kind="ExternalInput").ap()
    bv_d = nc.dram_tensor("bv", [2, H], f32, kind="ExternalInput").ap()
    wih_d = nc.dram_tensor("wih_t", [H, H3], f32, kind="ExternalInput").ap()
    whh_d = nc.dram_tensor("whh_t", [H, H3], f32, kind="ExternalInput").ap()
    # grub cols: 0: b_ih_r+b_hh_r, 1: b_ih_z+b_hh_z, 2: b_ih_n, 3: b_hh_n
    grub_d = nc.dram_tensor("grub", [H, 4], f32, kind="ExternalInput").ap()
    fcw_d = nc.dram_tensor("fcw", [H, A], f32, kind="ExternalInput").ap()
    fcb_d = nc.dram_tensor("fcb", [A], f32, kind="ExternalInput").ap()

    qout_d = nc.dram_tensor("qout", [BPC * N, A], f32, kind="ExternalOutput").ap()
    hout_d = nc.dram_tensor("hout", [BPC * N, H], f32, kind="ExternalOutput").ap()

    def bcast_ap(src):
        return bass.AP(
            tensor=src.tensor, offset=src.offset,
            ap=[[0, 128]] + [list(p) for p in src.ap],
        )

    with tile.TileContext(nc) as tc:
        with (
            tc.tile_pool(name="singles", bufs=1) as singles,
            tc.tile_pool(name="mn", bufs=4) as p_mn,          # mask natural f32
            tc.tile_pool(name="mb", bufs=4) as p_mb,          # mask natural bf16
            tc.tile_pool(name="mT", bufs=2) as p_mT,          # mask transposed bf16
            tc.tile_pool(name="xs", bufs=3) as p_xs,
            tc.tile_pool(name="hT", bufs=2) as p_hT,          # h1T/qT/kT/h2T/outT
            tc.tile_pool(name="v", bufs=NT + 2) as p_v,
            tc.tile_pool(name="e", bufs=3) as p_e,
            tc.tile_pool(name="p", bufs=3) as p_p,
            tc.tile_pool(name="h3", bufs=BPC) as p_h3,
            tc.tile_pool(name="r", bufs=2) as p_r,
            tc.tile_pool(name="gru", bufs=1) as p_gru,
            tc.tile_pool(name="out", bufs=4) as p_out,
            tc.tile_pool(name="pmm", bufs=2, space="PSUM") as pmm,
            tc.tile_pool(name="pacc", bufs=1, space="PSUM") as pacc,
            tc.tile_pool(name="prs", bufs=1, space="PSUM") as prs,
        ):
            masks = {}

            def emit_mask(b):
                # mask -> maskT (bf16) via 3D XBAR, one per row-tile
                mTa = p_mT.tile([128, NT, NP], bf16, tag="mT")
                for it in range(NT):
                    ts = _ts(it)
                    mn = p_mn.tile([128, NP], f32, tag="mn")
                    if ts < 128:
                        nc.vector.memset(mn, 0.0)
                    else:
                        nc.vector.memset(mn[:, N:NP], 0.0)
                    for ch in range(4):
                        r0, r1 = ch * 32, min((ch + 1) * 32, ts)
                        if r0 >= r1:
                            break
                        eng = nc.sync if ch % 2 == 0 else nc.scalar
                        eng.dma_start(
                            out=mn[r0:r1, :N],
                            in_=mask_d[b, it * 128 + r0 : it * 128 + r1, :],
                        )
                    mb = p_mb.tile([128, NP], bf16, tag="mb")
                    nc.vector.tensor_copy(out=mb, in_=mn)
                    eng = nc.sync if it % 2 == 0 else nc.scalar
                    eng.dma_start_transpose(
                        mTa[:, :, it * 128 : (it + 1) * 128], mb
                    )
                mT = [mTa[:, e, :] for e in range(NT)]
                # avoid 0-rowsum NaNs in padded query columns
                nc.vector.memset(mT[0][0:1, N:NP], 1.0)
                return mT

            # ---------- constants / weights ----------
            id_bf = singles.tile([128, 128], bf16)
            make_identity(nc, id_bf)
            id_f32 = singles.tile([128, 128], f32)
            make_identity(nc, id_f32)
            ones_bf = singles.tile([128, 128], bf16)
            nc.vector.memset(ones_bf, 1.0)

            encw_sb = singles.tile([DIN, H], bf16)
            nc.sync.dma_start(out=encw_sb, in_=encw_d)
            watt_sb = singles.tile([H, 2, 4, H], bf16)
            nc.sync.dma_start(out=watt_sb, in_=watt_d)
            attb_sb = singles.tile([H, 8], f32)
            nc.sync.dma_start(out=attb_sb, in_=attb_d)
            encb_sb = singles.tile([H, 1], f32)
            nc.sync.dma_start(out=encb_sb, in_=encb_d)
            bvb_sb = singles.tile([128, 2, H], f32)
            nc.sync.dma_start(out=bvb_sb, in_=bcast_ap(bv_d))
            wih_sb = singles.tile([H, H3], f32)
            nc.sync.dma_start(out=wih_sb, in_=wih_d)
            whh_sb = singles.tile([H, H3], f32)
            nc.sync.dma_start(out=whh_sb, in_=whh_d)
            wihr_sb = singles.tile([H, H3], f32r)
            nc.vector.tensor_copy(out=wihr_sb, in_=wih_sb)
            whhr_sb = singles.tile([H, H3], f32r)
            nc.vector.tensor_copy(out=whhr_sb, in_=whh_sb)
            grub_sb = singles.tile([H, 4], f32)
            nc.sync.dma_start(out=grub_sb, in_=grub_d)
            fcw_sb = singles.tile([H, A], f32)
            nc.sync.dma_start(out=fcw_sb, in_=fcw_d)
            fcb_sb = singles.tile([128, A], f32)
            nc.sync.dma_start(out=fcb_sb, in_=bcast_ap(fcb_d))

            h3Ts = {}

            # =================== attention (one batch) ===================
            def emit_attention(b):
                mT = masks.pop(b) if b in masks else emit_mask(b)

                # ---- x -> xT (bf16, padded cols zero) ----
                xT = p_xs.tile([DIN, NP], bf16, tag="xT")
                for it in range(NT):
                    ts = _ts(it)
                    xt = p_xs.tile([128, DIN], f32, tag="xt")
                    if ts < 128:
                        nc.vector.memset(xt, 0.0)
                    nc.sync.dma_start(
                        out=xt[:ts, :], in_=x_d[b, it * 128 : it * 128 + ts, :]
                    )
                    xb = p_xs.tile([128, DIN], bf16, tag="xb")
                    nc.vector.tensor_copy(out=xb, in_=xt)
                    pst = pmm.tile([DIN, NP], bf16, tag="mm")
                    nc.tensor.transpose(pst[:, :128], xb, id_bf)
                    nc.vector.tensor_copy(
                        out=xT[:, it * 128 : (it + 1) * 128], in_=pst[:, :128]
                    )

                # ---- h1T = relu(encW.T @ xT + encb) ----
                h1T = p_hT.tile([H, NP], bf16, tag="h1")
                ps = pmm.tile([128, NP], f32, tag="mm")
                for hf in range(2):
                    nc.tensor.matmul(
                        ps[:, hf * F : (hf + 1) * F], encw_sb,
                        xT[:, hf * F : (hf + 1) * F], start=True, stop=True,
                    )
                nc.scalar.activation(
                    out=h1T, in_=ps, func=AF.Relu, bias=encb_sb, scale=1.0
                )

                # ---- two attention layers ----
                hT = h1T
                for l in range(2):
                    qT = p_hT.tile([H, NP], bf16, tag="q")
                    kT = p_hT.tile([H, NP], bf16, tag="k")
                    for (dst, wi) in ((qT, 0), (kT, 1)):
                        ps = pmm.tile([128, NP], f32, tag="mm")
                        for hf in range(2):
                            nc.tensor.matmul(
                                ps[:, hf * F : (hf + 1) * F], watt_sb[:, l, wi, :],
                                hT[:, hf * F : (hf + 1) * F], start=True, stop=True,
                            )
                        nc.scalar.activation(
                            out=dst, in_=ps, func=AF.Relu,
                            bias=attb_sb[:, l * 4 + wi : l * 4 + wi + 1], scale=1.0,
                        )
                    # v in natural layout [node, H] (one tile per 128 nodes)
                    vs = []
                    for jt in range(NT):
                        ps = pmm.tile([128, NP], f32, tag="mm")
                        nc.tensor.matmul(
                            ps[:, :H], hT[:, jt * 128 : (jt + 1) * 128],
                            watt_sb[:, l, 2, :], start=True, stop=True,
                        )
                        vt = p_v.tile([128, H], f32, tag="vf")
                        nc.vector.scalar_tensor_tensor(
                            out=vt, in0=ps[:, :H], scalar=0.0,
                            in1=bvb_sb[:, l, :], op0=OP.bypass, op1=OP.add,
                        )
                        vb = p_v.tile([128, H], bf16, tag="vb")
                        nc.vector.tensor_scalar_max(vb, vt, 0.0)
                        vs.append(vb)

                    psA = pacc.tile([128, NP], f32, tag="acc")
                    psR = prs.tile([128, NP], f32, tag="rs")
                    for jt in range(NT):
                        psS = pmm.tile([128, NP], f32, tag="mm")
                        for hf in range(2):
                            nc.tensor.matmul(
                                psS[:, hf * F : (hf + 1) * F],
                                kT[:, jt * 128 : (jt + 1) * 128],
                                qT[:, hf * F : (hf + 1) * F],
                                start=True, stop=True,
                            )
                        et = p_e.tile([128, NP], bf16, tag="e")
                        nc.scalar.activation(
                            out=et, in_=psS, func=AF.Exp, bias=0.0, scale=1.0
                        )
                        pt = p_p.tile([128, NP], bf16, tag="p")
                        nc.vector.tensor_mul(pt, et, mT[jt])
                        for hf in range(2):
                            nc.tensor.matmul(
                                psA[:, hf * F : (hf + 1) * F], vs[jt],
                                pt[:, hf * F : (hf + 1) * F],
                                start=(jt == 0), stop=(jt == NT - 1),
                            )
                            nc.tensor.matmul(
                                psR[:, hf * F : (hf + 1) * F], ones_bf,
                                pt[:, hf * F : (hf + 1) * F],
                                start=(jt == 0), stop=(jt == NT - 1),
                            )
                    r_all = p_r.tile([128, NP], f32, tag="r")
                    r_scr = p_r.tile([128, NP], f32, tag="rscr", bufs=1)
                    outT = p_hT.tile([H, NP], bf16, tag="o")
                    nc.vector.reciprocal_approx_accurate(
                        out=r_all, in_=psR, scratch=r_scr
                    )
                    for hf in range(2):
                        sl = slice(hf * F, (hf + 1) * F)
                        nc.vector.tensor_mul(outT[:, sl], psA[:, sl], r_all[:, sl])
                    # h_next = relu(Wo.T @ outT + bo)
                    if l == 0:
                        hnext = p_hT.tile([H, NP], bf16, tag="h2")
                    else:
                        hnext = p_h3.tile([H, NP], f32r, tag="h3")
                    ps = pmm.tile([128, NP], f32, tag="mm")
                    for hf in range(2):
                        nc.tensor.matmul(
                            ps[:, hf * F : (hf + 1) * F], watt_sb[:, l, 3, :],
                            outT[:, hf * F : (hf + 1) * F], start=True, stop=True,
                        )
                    nc.scalar.activation(
                        out=hnext, in_=ps, func=AF.Relu,
                        bias=attb_sb[:, l * 4 + 3 : l * 4 + 4], scale=1.0,
                    )
                    hT = hnext
                h3Ts[b] = hT

            # =================== GRU (one batch, float32r matmuls) ===========
            def emit_hid_pipe(b):
                hinT = p_gru.tile([H, NP], f32r, tag="hinr", bufs=4)
                for it in range(NT):
                    ts = _ts(it)
                    ht_f = p_xs.tile([128, H], f32, tag="hidt")
                    if ts < 128:
                        nc.vector.memset(ht_f, 0.0)
                    nc.sync.dma_start(
                        out=ht_f[:ts, :],
                        in_=hid_d[b * N + it * 128 : b * N + it * 128 + ts, :],
                    )
                    pst = pmm.tile([128, NP], f32, tag="mm")
                    nc.tensor.transpose(pst[:, :128], ht_f, id_f32)
                    nc.vector.tensor_copy(
                        out=hinT[:, it * 128 : (it + 1) * 128], in_=pst[:, :128]
                    )
                return hinT

            def emit_gru(b, hinT):
                h3T = h3Ts[b]

                # r and z gates: sigmoid(W_ih_g.T@h3T + W_hh_g.T@hinT + b_g)
                gates = []
                for g in range(2):
                    gt = p_gru.tile([H, NP], f32, tag=f"g{g}", bufs=2)
                    ps = pmm.tile([128, NP], f32, tag="mm")
                    for hf in range(2):
                        sl = slice(hf * F, (hf + 1) * F)
                        nc.tensor.matmul(
                            ps[:, sl], wihr_sb[:, g * H : (g + 1) * H],
                            h3T[:, sl], start=True, stop=False,
                        )
                        nc.tensor.matmul(
                            ps[:, sl], whhr_sb[:, g * H : (g + 1) * H],
                            hinT[:, sl], start=False, stop=True,
                        )
                    nc.scalar.activation(
                        out=gt, in_=ps, func=AF.Sigmoid,
                        bias=grub_sb[:, g : g + 1], scale=1.0,
                    )
                    gates.append(gt)
                rg, zg = gates
                # n = tanh(xn + b_ihn + r*(hn + b_hhn))
                ng = p_gru.tile([H, NP], f32, tag="n", bufs=2)
                psX = pmm.tile([128, NP], f32, tag="mm")
                psHn = pmm.tile([128, NP], f32, tag="mm")
                for hf in range(2):
                    sl = slice(hf * F, (hf + 1) * F)
                    nc.tensor.matmul(
                        psX[:, sl], wihr_sb[:, 2 * H : 3 * H],
                        h3T[:, sl], start=True, stop=True,
                    )
                    nc.tensor.matmul(
                        psHn[:, sl], whhr_sb[:, 2 * H : 3 * H],
                        hinT[:, sl], start=True, stop=True,
                    )
                tt = p_gru.tile([H, NP], f32, tag="t")
                pre = p_gru.tile([H, NP], f32, tag="pre")
                for hf in range(2):
                    sl = slice(hf * F, (hf + 1) * F)
                    nc.vector.scalar_tensor_tensor(
                        out=tt[:, sl], in0=psHn[:, sl], scalar=grub_sb[:, 3:4],
                        in1=rg[:, sl], op0=OP.add, op1=OP.mult,
                    )
                    nc.vector.scalar_tensor_tensor(
                        out=pre[:, sl], in0=psX[:, sl], scalar=grub_sb[:, 2:3],
                        in1=tt[:, sl], op0=OP.add, op1=OP.add,
                    )
                nc.scalar.activation(
                    out=ng, in_=pre, func=AF.Tanh, bias=0.0, scale=1.0
                )
                # h = n + z*(hin - n)
                dd = p_gru.tile([H, NP], f32, tag="d")
                nc.vector.tensor_sub(dd, hinT, ng)
                nc.vector.tensor_mul(dd, dd, zg)
                hnT = p_gru.tile([H, NP], f32, tag="hn", bufs=2)
                nc.vector.tensor_add(hnT, dd, ng)

                # outputs
                for it in range(NT):
                    ts = _ts(it)
                    sl = slice(it * 128, it * 128 + 128)
                    psQ = pmm.tile([128, NP], f32, tag="mm")
                    nc.tensor.matmul(
                        psQ[:, :A], hnT[:, sl],
                        fcw_sb, start=True, stop=True,
                    )
                    qo = p_out.tile([128, A], f32, tag="qo")
                    nc.vector.tensor_add(qo, psQ[:, :A], fcb_sb)
                    nc.sync.dma_start(
                        out=qout_d[b * N + it * 128 : b * N + it * 128 + ts, :],
                        in_=qo[:ts, :],
                    )
                    psT = pmm.tile([128, NP], f32, tag="mm")
                    nc.tensor.transpose(psT[:, :128], hnT[:, sl], id_f32)
                    hsb = p_out.tile([128, H], f32, tag="ho")
                    nc.vector.tensor_copy(out=hsb, in_=psT[:, :128])
                    nc.sync.dma_start(
                        out=hout_d[b * N + it * 128 : b * N + it * 128 + ts, :],
                        in_=hsb[:ts, :],
                    )

            # ===== driver: attention (+hid prefetch) phase, then GRU phase ===
            hins = {}
            for b in range(BPC):
                emit_attention(b)
                hins[b] = emit_hid_pipe(b)
            for b in range(BPC):
                emit_gru(b, hins.pop(b))

    nc.compile()
    return nc


@functools.lru_cache(maxsize=1)
def _get_program():
    return build_program()


def _prep_weights(inputs):
    bf = ml_dtypes.bfloat16
    f = np.float32
    out = {}
    out["encw"] = np.ascontiguousarray(inputs["enc_W"], dtype=bf)
    watt = np.stack(
        [
            np.stack([inputs["Wq1"], inputs["Wk1"], inputs["Wv1"], inputs["Wo1"]]),
            np.stack([inputs["Wq2"], inputs["Wk2"], inputs["Wv2"], inputs["Wo2"]]),
        ]
    )
    out["watt"] = np.ascontiguousarray(watt.transpose(2, 0, 1, 3), dtype=bf)
    attb = np.stack(
        [
            inputs["bq1"], inputs["bk1"], inputs["bv1"], inputs["bo1"],
            inputs["bq2"], inputs["bk2"], inputs["bv2"], inputs["bo2"],
        ],
        axis=1,
    )  # [128, 8]
    out["attb"] = np.ascontiguousarray(attb, dtype=f)
    out["encb"] = np.ascontiguousarray(inputs["enc_b"][:, None], dtype=f)
    out["bv"] = np.ascontiguousarray(
        np.stack([inputs["bv1"], inputs["bv2"]]), dtype=f
    )
    out["wih_t"] = np.ascontiguousarray(inputs["W_ih"].T, dtype=f)
    out["whh_t"] = np.ascontiguousarray(inputs["W_hh"].T, dtype=f)
    b_ih, b_hh = inputs["b_ih"], inputs["b_hh"]
    grub = np.stack(
        [
            b_ih[0:H] + b_hh[0:H],
            b_ih[H : 2 * H] + b_hh[H : 2 * H],
            b_ih[2 * H : 3 * H],
            b_hh[2 * H : 3 * H],
        ],
        axis=1,
    )  # [128, 4]
    out["grub"] = np.ascontiguousarray(grub, dtype=f)
    out["fcw"] = np.ascontiguousarray(inputs["fc_W"], dtype=f)
    out["fcb"] = np.ascontiguousarray(inputs["fc_b"], dtype=f)
    return out


def run(inputs, trace=False):
    nc = _get_program()
    w = _prep_weights(inputs)
    x = np.asarray(inputs["x"], dtype=np.float32)
    mask = np.asarray(inputs["mask"], dtype=np.float32)
    hid = np.asarray(inputs["hidden_state"], dtype=np.float32)
    in_maps = []
    for c in range(NCORES):
        bs = slice(c * BPC, (c + 1) * BPC)
        m = dict(w)
        m["x"] = np.ascontiguousarray(x[bs])
        m["mask"] = np.ascontiguousarray(mask[bs])
        m["hid"] = np.ascontiguousarray(hid[c * BPC * N : (c + 1) * BPC * N])
        in_maps.append(m)
    res = bass_utils.run_bass_kernel_spmd(
        nc, in_maps, core_ids=list(range(NCORES)), trace=trace
    )
    q_out = np.concatenate([r["qout"] for r in res.results], axis=0)
    h_out = np.concatenate([r["hout"] for r in res.results], axis=0)
    return (q_out, h_out), res


def kernel(**inputs):
    (q_out, h_out), _ = run(inputs)
    return q_out, h_out
